# revision 29
# baseline (speedup 1.0000x reference)
"""MoE-routed transformer encoder layer on 8 Trainium2 cores.

Routing (mean -> nearest center -> expert id) is computed on host; sentences
are dispatched to cores so that each core runs exactly one expert's weights
over its share of sentences (expert/data parallelism, no device collectives).

Device kernel: dense encoder layer QKV -> attention -> out-proj -> LN1 ->
FFN(gelu) -> LN2. Weights and matmul operands are bf16 (full-rate PE, half
DMA/SBUF); PSUM accumulation, layernorm and softmax statistics stay fp32.
Weights are DMA'd once per phase and stay resident in SBUF across groups.
"""

import numpy as np

H = 768
NH = 12
HD = 64
FF = 3072
S = 128
E = 4
EPS = 1e-12
NCORES = 8

PARAM_KEYS = [
    "wq", "wk", "wv", "wo", "bq", "bk", "bv", "bo",
    "ln1_g", "ln1_b", "w1", "b1", "w2", "b2", "ln2_g", "ln2_b",
]
BF16_KEYS = {"wq", "wk", "wv", "wo", "w1", "w2"}

_BUILD_CACHE = {}
LAST_RUN_WALL_NS = None
LAST_RESULT = None  # BassKernelResults of the most recent run (for profiling)


def _build(nslot, use_mask):
    import concourse.bass as bass
    import concourse.mybir as mybir
    import concourse.tile as tile
    from concourse import bacc
    from concourse.masks import make_identity

    f32 = mybir.dt.float32
    bf16 = mybir.dt.bfloat16

    NS = nslot
    assert NS % 4 == 0
    G = NS // 4

    nc = bacc.Bacc("TRN2", target_bir_lowering=False, debug=False)

    x_d = nc.dram_tensor("x", [NS, S, H], f32, kind="ExternalInput").ap()
    mask_d = nc.dram_tensor("mask", [NS, S], f32, kind="ExternalInput").ap()
    wq_d = nc.dram_tensor("wq", [H, H], bf16, kind="ExternalInput").ap()
    wk_d = nc.dram_tensor("wk", [H, H], bf16, kind="ExternalInput").ap()
    wv_d = nc.dram_tensor("wv", [H, H], bf16, kind="ExternalInput").ap()
    wo_d = nc.dram_tensor("wo", [H, H], bf16, kind="ExternalInput").ap()
    bq_d = nc.dram_tensor("bq", [H], f32, kind="ExternalInput").ap()
    bk_d = nc.dram_tensor("bk", [H], f32, kind="ExternalInput").ap()
    bv_d = nc.dram_tensor("bv", [H], f32, kind="ExternalInput").ap()
    bo_d = nc.dram_tensor("bo", [H], f32, kind="ExternalInput").ap()
    g1_d = nc.dram_tensor("ln1_g", [H], f32, kind="ExternalInput").ap()
    b1l_d = nc.dram_tensor("ln1_b", [H], f32, kind="ExternalInput").ap()
    w1_d = nc.dram_tensor("w1", [H, FF], bf16, kind="ExternalInput").ap()
    b1_d = nc.dram_tensor("b1", [FF], f32, kind="ExternalInput").ap()
    w2_d = nc.dram_tensor("w2", [FF, H], bf16, kind="ExternalInput").ap()
    b2_d = nc.dram_tensor("b2", [H], f32, kind="ExternalInput").ap()
    g2_d = nc.dram_tensor("ln2_g", [H], f32, kind="ExternalInput").ap()
    b2l_d = nc.dram_tensor("ln2_b", [H], f32, kind="ExternalInput").ap()
    out_d = nc.dram_tensor("out", [NS, S, H], bf16, kind="ExternalOutput").ap()

    x_sv = x_d.rearrange("n s h -> s n h")       # partition dim = sequence pos
    out_sv = out_d.rearrange("n s h -> s n h")

    with tile.TileContext(nc) as tc:
        _kernel_body(
            nc, tc, bass, mybir, tile, make_identity, NS, G, use_mask,
            x_sv, out_sv, mask_d,
            wq_d, wk_d, wv_d, wo_d, bq_d, bk_d, bv_d, bo_d,
            g1_d, b1l_d, w1_d, b1_d, w2_d, b2_d, g2_d, b2l_d,
        )
    nc.compile()
    return nc


def _kernel_body(nc, tc, bass, mybir, tile, make_identity, NS, G, use_mask,
                 x_sv, out_sv, mask_d,
                 wq_d, wk_d, wv_d, wo_d, bq_d, bk_d, bv_d, bo_d,
                 g1_d, b1l_d, w1_d, b1_d, w2_d, b2_d, g2_d, b2l_d):
    f32 = mybir.dt.float32
    bf16 = mybir.dt.bfloat16
    AF = mybir.ActivationFunctionType
    ALU = mybir.AluOpType

    with (
        tc.tile_pool(name="const", bufs=1) as constp,
        tc.tile_pool(name="ybuf", bufs=1) as ybufp,
    ):
        ident = constp.tile([128, 128], f32)
        make_identity(nc, ident)
        eps_t = constp.tile([128, 1], f32)
        nc.vector.memset(eps_t, EPS)
        b1_sb = constp.tile([128, 24], f32)
        nc.gpsimd.dma_start(b1_sb, b1_d.rearrange("(o p) -> p o", p=128))

        def repl(pool, src, nm):
            t = pool.tile([128, H], f32, tag=nm, name=nm)
            bsrc = bass.AP(
                tensor=src.tensor, offset=src.offset, ap=[[0, 128], [1, H]]
            )
            nc.gpsimd.dma_start(t, bsrc)
            return t

        b2_r = repl(constp, b2_d, "b2_r")
        g2_r = repl(constp, g2_d, "g2_r")
        b2l_r = repl(constp, b2l_d, "b2l_r")
        y_all = ybufp.tile([128, NS, H], f32)
        yT_all = ybufp.tile([128, 6, NS, 128], bf16)

        # ---------------- Phase A: attention + LN1 -> y_all --------------
        with (
            tc.tile_pool(name="pa", bufs=1) as pa,
            tc.tile_pool(name="pa2", bufs=2) as pa2,
            tc.tile_pool(name="px", bufs=2) as px,
            tc.tile_pool(name="psA", bufs=4, space="PSUM") as psA,
            tc.tile_pool(name="psVO", bufs=2, space="PSUM") as psVO,
        ):
            bq_sb = pa.tile([128, 6], f32, tag="bq_sb", name="bq_sb")
            nc.gpsimd.dma_start(bq_sb, bq_d.rearrange("(o p) -> p o", p=128))
            bk_sb = pa.tile([128, 6], f32, tag="bk_sb", name="bk_sb")
            nc.gpsimd.dma_start(bk_sb, bk_d.rearrange("(o p) -> p o", p=128))
            bv_r = repl(pa, bv_d, "bv_r")
            bo_r = repl(pa, bo_d, "bo_r")
            g1_r = repl(pa, g1_d, "g1_r")
            b1l_r = repl(pa, b1l_d, "b1l_r")

            # per-phase resident weights (bf16, loaded once)
            wq_sb = pa.tile([128, 6, H], bf16, tag="wq_sb", name="wq_sb")
            nc.sync.dma_start(wq_sb, wq_d.rearrange("(ko p) m -> p ko m", p=128))
            wk_sb = pa.tile([128, 6, H], bf16, tag="wk_sb", name="wk_sb")
            nc.sync.dma_start(wk_sb, wk_d.rearrange("(ko p) m -> p ko m", p=128))
            wv_sb = pa.tile([128, 6, H], bf16, tag="wv_sb", name="wv_sb")
            nc.sync.dma_start(wv_sb, wv_d.rearrange("(ko p) m -> p ko m", p=128))
            wo_sb = pa.tile([128, 6, H], bf16, tag="wo_sb", name="wo_sb")
            nc.sync.dma_start(wo_sb, wo_d.rearrange("(ko p) m -> p ko m", p=128))

            for g in range(G):
                s0 = g * 4
                x_g = px.tile([128, 4, H], f32, tag="x_g")
                # ACT HWDGE queue: not behind the 4.7MB of weights on sync
                nc.scalar.dma_start(x_g, x_sv[:, s0 : s0 + 4, :])
                if use_mask:
                    mrep = px.tile([128, 4, S], f32, tag="mrep")
                    src = bass.AP(
                        tensor=mask_d.tensor,
                        offset=s0 * S,
                        ap=[[0, 128], [S, 4], [1, S]],
                    )
                    nc.gpsimd.dma_start(mrep, src)

                # x transposed: xT[p, c, si, s] = x[s, si, c*128+p]
                xT = pa.tile([128, 6, 4, 128], bf16, tag="xT")
                for c in range(6):
                    pt4 = psA.tile([128, 512], f32, tag="pq", name="pt4")
                    for si in range(4):
                        nc.tensor.transpose(
                            pt4[:, si * 128 : (si + 1) * 128],
                            x_g[:, si, c * 128 : (c + 1) * 128],
                            ident,
                        )
                    nc.scalar.activation(xT[:, c, :, :], pt4, AF.Identity)

                # qT/kT: weight-stationary over 4-sentence pack (N=512)
                qT = pa.tile([128, 6, 4, 128], bf16, tag="qT")
                kT = pa.tile([128, 6, 4, 128], bf16, tag="kT")
                for w_sb, bias_sb, dstT in (
                    (wq_sb, bq_sb, qT),
                    (wk_sb, bk_sb, kT),
                ):
                    for mc in range(6):
                        pq = psA.tile([128, 512], f32, tag="pq", name="pq")
                        for kc in range(6):
                            nc.tensor.matmul(
                                pq,
                                w_sb[:, kc, mc * 128 : (mc + 1) * 128],
                                xT[:, kc, :, :],
                                start=(kc == 0),
                                stop=(kc == 5),
                            )
                        nc.scalar.activation(
                            dstT[:, mc, :, :],
                            pq,
                            AF.Identity,
                            bias=bias_sb[:, mc : mc + 1],
                            scale=1.0,
                        )

                # v in natural layout [s, 768]
                v_g = pa.tile([128, 4, H], bf16, tag="v_g")
                for si in range(4):
                    pv = psVO.tile([128, H], f32, tag="pv")
                    for c0, c1 in ((0, 512), (512, H)):
                        for kc in range(6):
                            nc.tensor.matmul(
                                pv[:, c0:c1],
                                xT[:, kc, si, :],
                                wv_sb[:, kc, c0:c1],
                                start=(kc == 0),
                                stop=(kc == 5),
                            )
                    nc.vector.tensor_add(v_g[:, si, :], pv, bv_r)

                # attention per sentence
                ctxT = pa.tile([128, 6, 4, 128], bf16, tag="xT")  # reuse xT slot
                for si in range(4):
                    attn = pa2.tile([128, NH, S], f32, tag="attn")
                    sums = pa2.tile([128, NH], f32, tag="sums")
                    for h in range(NH):
                        # one PSUM bank per head (PE-write while ACT-reads a
                        # shared bank is fatal on HW); head pairs pack into
                        # the PE array via tile_position and run concurrently
                        psc = psA.tile([128, 128], f32, tag="pq", name="psc")
                        nc.tensor.matmul(
                            psc,
                            qT[(h % 2) * 64 : (h % 2) * 64 + 64, h // 2, si, :],
                            kT[(h % 2) * 64 : (h % 2) * 64 + 64, h // 2, si, :],
                            start=True,
                            stop=True,
                            tile_position=((h % 2) * 64, 0),
                        )
                        if use_mask:
                            tmp = pa.tile([128, S], f32, tag="msk_tmp")
                            nc.vector.tensor_scalar_mul(tmp, psc, 0.125)
                            nc.vector.tensor_add(tmp, tmp, mrep[:, si, :])
                            nc.scalar.activation(
                                attn[:, h, :], tmp, AF.Exp,
                                bias=0.0, scale=1.0,
                                accum_out=sums[:, h : h + 1],
                            )
                        else:
                            nc.scalar.activation(
                                attn[:, h, :], psc, AF.Exp,
                                bias=0.0, scale=0.125,
                                accum_out=sums[:, h : h + 1],
                            )
                    rs = pa2.tile([128, NH], f32, tag="rs")
                    nc.vector.reciprocal(rs, sums)
                    for h in range(NH):
                        # split across DVE and Pool: halves the softmax-chain
                        # latency that gates the attnT transposes on PE
                        eng = nc.vector if h % 2 == 0 else nc.gpsimd
                        eng.tensor_scalar_mul(
                            attn[:, h, :], attn[:, h, :], rs[:, h : h + 1]
                        )
                    attnT = pa2.tile([128, NH, S], bf16, tag="attnT")
                    for hg in range(3):
                        pt4 = psA.tile([128, 512], f32, tag="pq", name="pt4")
                        for j in range(4):
                            nc.tensor.transpose(
                                pt4[:, j * 128 : (j + 1) * 128],
                                attn[:, hg * 4 + j, :],
                                ident,
                            )
                        nc.scalar.activation(
                            attnT[:, hg * 4 : hg * 4 + 4, :], pt4, AF.Identity
                        )
                    for hq in range(2):  # 3 head-pairs per psum tile
                        pc3 = psA.tile([128, 512], f32, tag="pq", name="pc3")
                        for jp in range(3):
                            hp = hq * 3 + jp
                            nc.tensor.matmul(
                                pc3[0:64, jp * 128 : (jp + 1) * 128],
                                v_g[:, si, (2 * hp) * 64 : (2 * hp + 1) * 64],
                                attnT[:, 2 * hp, :],
                                start=True, stop=True,
                                tile_position=(0, 0),
                            )
                            nc.tensor.matmul(
                                pc3[64:128, jp * 128 : (jp + 1) * 128],
                                v_g[:, si, (2 * hp + 1) * 64 : (2 * hp + 2) * 64],
                                attnT[:, 2 * hp + 1, :],
                                start=True, stop=True,
                                tile_position=(0, 64),
                            )
                        nc.vector.tensor_copy(
                            ctxT[:, hq * 3 : hq * 3 + 3, si, :],
                            pc3[:, 0:384],
                        )

                # out-proj + bo + residual + LN1 -> y_all
                for si in range(4):
                    po = psVO.tile([128, H], f32, tag="pv")
                    for c0, c1 in ((0, 512), (512, H)):
                        for kc in range(6):
                            nc.tensor.matmul(
                                po[:, c0:c1],
                                ctxT[:, kc, si, :],
                                wo_sb[:, kc, c0:c1],
                                start=(kc == 0), stop=(kc == 5),
                            )
                    z = pa2.tile([128, H], f32, tag="z")
                    nc.vector.tensor_add(z, po, bo_r)
                    nc.vector.tensor_add(z, z, x_g[:, si, :])
                    # LN1 (stats in f32)
                    st = pa2.tile([128, 3, 6], f32, tag="st")
                    zv = z.rearrange("p (a b) -> p a b", a=3)
                    for i in range(3):
                        nc.vector.bn_stats(st[:, i, :], zv[:, i, :])
                    mv = pa2.tile([128, 2], f32, tag="mv")
                    nc.vector.bn_aggr(mv, st)
                    sd = pa2.tile([128, 1], f32, tag="sd")
                    nc.scalar.activation(
                        sd, mv[:, 1:2], AF.Sqrt, bias=eps_t[:, 0:1], scale=1.0
                    )
                    nc.vector.reciprocal(sd, sd)
                    nm = pa2.tile([128, 1], f32, tag="nm")
                    nc.vector.tensor_mul(nm, mv[:, 0:1], sd)
                    nc.vector.tensor_scalar_mul(nm, nm, -1.0)
                    y_f = pa2.tile([128, H], f32, tag="y_f")
                    nc.scalar.activation(
                        y_f, z, AF.Identity, bias=nm[:, 0:1], scale=sd[:, 0:1]
                    )
                    nc.gpsimd.tensor_mul(y_f, y_f, g1_r)
                    yslot = y_all[:, s0 + si, :]
                    nc.gpsimd.tensor_add(yslot, y_f, b1l_r)
                    for ch in range(2):
                        pt3 = psA.tile([128, 512], f32, tag="pq", name="pt3")
                        for j in range(3):
                            c = ch * 3 + j
                            nc.tensor.transpose(
                                pt3[:, j * 128 : (j + 1) * 128],
                                yslot[:, c * 128 : (c + 1) * 128],
                                ident,
                            )
                        nc.scalar.activation(
                            yT_all[:, ch * 3 : ch * 3 + 3, s0 + si, :],
                            pt3[:, 0:384],
                            AF.Identity,
                        )

        # ---------------- Phase B: FFN + LN2 -> out ----------------------
        with (
            tc.tile_pool(name="pb", bufs=1) as pb,
            tc.tile_pool(name="pb2", bufs=2) as pb2,
            tc.tile_pool(name="psBg", bufs=2, space="PSUM") as psBg,
            tc.tile_pool(name="psBw", bufs=2, space="PSUM") as psBw,
        ):
            w1_sb = pb.tile([128, 6, FF], bf16, tag="w1_sb", name="w1_sb")
            nc.sync.dma_start(w1_sb, w1_d.rearrange("(ko p) f -> p ko f", p=128))
            w2_sb = pb.tile([128, 24, H], bf16, tag="w2_sb", name="w2_sb")
            nc.sync.dma_start(w2_sb, w2_d.rearrange("(ko p) h -> p ko h", p=128))

            for g in range(G):
                s0 = g * 4
                # w1 + gelu for the whole group: gT [128, 24, 4*128]
                gT = pb.tile([128, 24, 512], bf16, tag="gT")
                for fg in range(24):
                    pg = psBg.tile([128, 512], f32, tag="pg")
                    for kc in range(6):
                        nc.tensor.matmul(
                            pg,
                            w1_sb[:, kc, fg * 128 : (fg + 1) * 128],
                            yT_all[:, kc, s0 : s0 + 4, :],
                            start=(kc == 0), stop=(kc == 5),
                        )
                    nc.scalar.activation(
                        gT[:, fg, :], pg, AF.Gelu_apprx_tanh,
                        bias=b1_sb[:, fg : fg + 1], scale=1.0,
                    )

                for si in range(4):
                    pw2 = psBw.tile([128, H], f32, tag="pw2")
                    for c0, c1 in ((0, 512), (512, H)):
                        for kc in range(24):
                            nc.tensor.matmul(
                                pw2[:, c0:c1],
                                gT[:, kc, si * 128 : (si + 1) * 128],
                                w2_sb[:, kc, c0:c1],
                                start=(kc == 0), stop=(kc == 23),
                            )
                    z2 = pb2.tile([128, H], f32, tag="z2")
                    nc.vector.tensor_add(z2, pw2, b2_r)
                    nc.vector.tensor_add(z2, z2, y_all[:, s0 + si, :])
                    st = pb2.tile([128, 3, 6], f32, tag="stB")
                    z2v = z2.rearrange("p (a b) -> p a b", a=3)
                    for i in range(3):
                        nc.vector.bn_stats(st[:, i, :], z2v[:, i, :])
                    mv = pb2.tile([128, 2], f32, tag="mvB")
                    nc.vector.bn_aggr(mv, st)
                    sd = pb2.tile([128, 1], f32, tag="sdB")
                    nc.scalar.activation(
                        sd, mv[:, 1:2], AF.Sqrt, bias=eps_t[:, 0:1], scale=1.0
                    )
                    nc.vector.reciprocal(sd, sd)
                    nm = pb2.tile([128, 1], f32, tag="nmB")
                    nc.vector.tensor_mul(nm, mv[:, 0:1], sd)
                    nc.vector.tensor_scalar_mul(nm, nm, -1.0)
                    o_f = pb2.tile([128, H], f32, tag="o_f")
                    nc.scalar.activation(
                        o_f, z2, AF.Identity, bias=nm[:, 0:1], scale=sd[:, 0:1]
                    )
                    nc.gpsimd.tensor_mul(o_f, o_f, g2_r)
                    o = pb2.tile([128, H], bf16, tag="o")
                    nc.vector.tensor_add(o, o_f, b2l_r)
                    nc.scalar.dma_start(out_sv[:, s0 + si, :], o)


def _route_and_assign(hidden_states, centers):
    hp = hidden_states.mean(axis=1)  # [B, H]
    d2 = (
        (hp * hp).sum(-1, keepdims=True)
        - 2.0 * hp @ centers.T
        + (centers * centers).sum(-1)[None, :]
    )
    eid = np.argmin(d2, axis=1)  # [B]
    B = eid.shape[0]
    counts = np.bincount(eid, minlength=E)
    active = [e for e in range(E) if counts[e] > 0]
    # apportion cores to active experts proportionally (min 1 each)
    cores_e = {e: 1 for e in active}
    rem = NCORES - len(active)
    if rem > 0:
        quota = {e: counts[e] * NCORES / B for e in active}
        frac = {e: quota[e] - 1 for e in active}
        order = sorted(active, key=lambda e: -frac[e])
        whole = {e: max(0, int(np.floor(frac[e]))) for e in active}
        used = sum(whole.values())
        while used > rem:  # trim if overflow
            for e in sorted(active, key=lambda e: -whole[e]):
                if used <= rem:
                    break
                if whole[e] > 0:
                    whole[e] -= 1
                    used -= 1
        for e in active:
            cores_e[e] += whole[e]
        rem -= used
        i = 0
        frac_order = sorted(active, key=lambda e: -(frac[e] - whole[e]))
        while rem > 0:
            cores_e[frac_order[i % len(frac_order)]] += 1
            rem -= 1
            i += 1
    # assign sentences of each expert round-robin over its cores
    assign = [[] for _ in range(NCORES)]  # core -> list of batch idx
    core_expert = [active[0] if active else 0] * NCORES
    next_core = 0
    for e in active:
        ncr = cores_e[e]
        idxs = np.nonzero(eid == e)[0]
        chunks = np.array_split(idxs, ncr)
        for ch in chunks:
            assign[next_core] = list(ch)
            core_expert[next_core] = e
            next_core += 1
    max_load = max(len(a) for a in assign)
    nslot = max(4, int(np.ceil(max_load / 4.0)) * 4)
    return assign, core_expert, nslot


def kernel(**inputs):
    global LAST_RUN_WALL_NS, LAST_RESULT
    import time

    import ml_dtypes
    from concourse.bass_utils import run_bass_kernel_spmd

    bf16 = ml_dtypes.bfloat16
    inputs = {k: np.ascontiguousarray(np.asarray(v)) for k, v in inputs.items()}
    hs = inputs["hidden_states"].astype(np.float32, copy=False)
    am = inputs["attention_mask"].astype(np.float32, copy=False)
    centers = inputs["centers"].astype(np.float32, copy=False)
    B = hs.shape[0]

    assign, core_expert, nslot = _route_and_assign(hs, centers)
    use_mask = bool(np.any(am != 0.0))

    key = (nslot, use_mask)
    if key not in _BUILD_CACHE:
        _BUILD_CACHE[key] = _build(nslot, use_mask)
    nc = _BUILD_CACHE[key]

    # convert each expert's big weights to bf16 once (reused by its cores)
    wcast = {
        k: [np.ascontiguousarray(inputs[k][e].astype(bf16)) for e in range(E)]
        for k in PARAM_KEYS if k in BF16_KEYS
    }
    in_maps = []
    for c in range(NCORES):
        e = core_expert[c]
        idxs = assign[c]
        x = np.zeros((nslot, S, H), np.float32)
        m = np.zeros((nslot, S), np.float32)
        for j, b in enumerate(idxs):
            x[j] = hs[b]
            m[j] = am[b]
        im = {"x": x, "mask": m}
        for k in PARAM_KEYS:
            if k in BF16_KEYS:
                im[k] = wcast[k][e]
            else:
                im[k] = np.ascontiguousarray(inputs[k][e])
        in_maps.append(im)

    t0 = time.perf_counter_ns()
    res = run_bass_kernel_spmd(nc, in_maps, core_ids=list(range(NCORES)))
    LAST_RUN_WALL_NS = time.perf_counter_ns() - t0
    LAST_RESULT = res

    out = np.zeros((B, S, H), np.float32)
    for c in range(NCORES):
        oc = res.results[c]["out"]
        for j, b in enumerate(assign[c]):
            out[b] = oc[j].astype(np.float32)
    return out
